# revision 21
# baseline (speedup 1.0000x reference)
"""Trainium2 Bass kernel for sigmoid-projection strictly-causal attention.

Reference computation (B=8, S=2048, D=512, U=512):
    q = sigmoid(x @ Wq); k = sigmoid(x @ Wv); v = sigmoid(x @ Wk)
    score = (q @ k^T) / sqrt(D)                       [S, S]
    mask: strictly causal (key j < query i); row 0 -> zeros
    out = softmax(score) @ v                          [S, U]

Sharding: data-parallel over batch, one batch element per NeuronCore (8
cores). Weights replicated. No collectives.

Per-core kernel (all matmuls in bf16; sigmoid/exp in f32 on ScalarE with
f32 PSUM accumulation):
  1. X and W loaded with casting SWDGE DMAs (f32 DRAM -> bf16 SBUF);
     X^T built via PE transposes (d on partitions).
  2. Q^T, K^T = sigmoid(W^T X^T) with u on partitions; V = sigmoid(X W)
     natural [s, u].  Sigmoid fused into the PSUM->SBUF eviction.
  3. Per 128-row query tile i: scores over keys [0, (i+1)*128) only
     (u-outer loop so each Q^T block's LDWEIGHTS is reused across key
     chunks), exp with the 1/sqrt(D) scale folded in (no max-subtraction
     needed since scores are bounded by sqrt(D)), strict-lower mask on
     the diagonal block, DVE row-sum for the denominator, DMA-transposed
     P blocks feed the P @ V accumulation, and the reciprocal
     denominator is applied on the PSUM->SBUF eviction.
"""

import sys

for _p in ("/opt/trn_rl_repo",):
    if _p not in sys.path:
        sys.path.insert(0, _p)

import numpy as np

B, S, D, U = 8, 2048, 512, 512
FP8_PROJ = True  # projections in fp8e4m3 with DoubleRow (2x PE throughput)
P = 128
NCORES = 8
DT = D // P  # 4 d-tiles
UT = U // P  # 4 u-tiles
ST = S // P  # 16 s-tiles
SC = S // 512  # 4 s-chunks

_cache = {}


def _build():
    import ml_dtypes
    import concourse.mybir as mybir
    import concourse.tile as tile
    from concourse import bacc

    f32 = mybir.dt.float32
    bf16 = mybir.dt.bfloat16
    AF = mybir.ActivationFunctionType
    ALU = mybir.AluOpType
    AX = mybir.AxisListType

    nc = bacc.Bacc("TRN2", target_bir_lowering=False, debug=False,
                   num_devices=NCORES)

    x_ext = nc.dram_tensor("query", [S, D], f32, kind="ExternalInput")
    wq_ext = nc.dram_tensor("Wq", [D, U], f32, kind="ExternalInput")
    wv_ext = nc.dram_tensor("Wv", [D, U], f32, kind="ExternalInput")
    wk_ext = nc.dram_tensor("Wk", [D, U], f32, kind="ExternalInput")
    out_ext = nc.dram_tensor("out", [S, U], f32, kind="ExternalOutput")

    ident_dram = nc.inline_tensor(
        np.eye(P, dtype=ml_dtypes.bfloat16), "ident_const")
    # [sq_p, sk_f] diagonal block additive mask: 0 where sk < sq (keep),
    # -1e30 elsewhere, applied to the f32 scores before exp.
    mask_dram = nc.inline_tensor(
        np.where(np.tril(np.ones((P, P), bool), -1), 0.0, -1e30)
        .astype(np.float32), "mask_const")

    inv_sqrt_d = 1.0 / float(np.sqrt(D))

    with tile.TileContext(nc) as tc:
        with (
            tc.tile_pool(name="const", bufs=1) as constp,
            tc.tile_pool(name="wpool", bufs=1) as wpool,
            tc.tile_pool(name="xfp", bufs=6) as xfp,
            tc.tile_pool(name="xbp", bufs=6) as xbp,
            tc.tile_pool(name="wfp", bufs=4) as wfp,
            tc.tile_pool(name="persist", bufs=1) as persist,
            tc.tile_pool(name="pp", bufs=3) as pp,
            tc.tile_pool(name="ptp", bufs=12) as ptp,
            tc.tile_pool(name="outp", bufs=3) as outp,
            tc.tile_pool(name="small", bufs=8) as smallp,
            tc.tile_pool(name="tps", bufs=3, space="PSUM") as tps,
            tc.tile_pool(name="mpsum", bufs=3, space="PSUM") as mpsum,
            tc.tile_pool(name="opsum", bufs=2, space="PSUM") as opsum,
        ):
            f8 = mybir.dt.float8e4
            DR = mybir.MatmulPerfMode.DoubleRow

            ident = constp.tile([P, P], bf16)
            nc.scalar.dma_start(ident[:], ident_dram[:])
            diag_mask = constp.tile([P, P], f32)
            nc.scalar.dma_start(diag_mask[:], mask_dram[:])

            # ---- input DMA schedule: first X chunk, then Wq/Wv, then the
            # remaining X tiles, interleaved across both HWDGE queues ----
            xf = [xfp.tile([P, D], f32, tag="xf", name=f"xf{st}")
                  for st in range(ST)]
            wstg = {}
            for name, ext in (("q", wq_ext), ("v", wv_ext)):
                for t in range(DT):
                    wstg[(name, t)] = (
                        wfp.tile([P, U], f32, tag="wf",
                                 name=f"wf_{name}_{t}"), ext)

            def dma_x(st):
                qeng = nc.sync if st % 2 == 0 else nc.scalar
                qeng.dma_start(xf[st][:], x_ext[st * P:(st + 1) * P, :])

            for st in range(4):
                dma_x(st)
            for name in ("q", "v"):
                for t in range(DT):
                    wf, ext = wstg[(name, t)]
                    qeng = nc.sync if t % 2 == 0 else nc.scalar
                    qeng.dma_start(wf[:], ext[t * P:(t + 1) * P, :])
            for st in range(4, ST):
                dma_x(st)

            # weights -> packed fp8 [P, DT, U]
            w_f8 = {}
            for name in ("q", "v"):
                w8 = wpool.tile([P, DT, U], f8, tag=f"w8_{name}",
                                name=f"w8_{name}")
                for t in range(DT):
                    nc.vector.tensor_copy(out=w8[:, t, :],
                                          in_=wstg[(name, t)][0][:])
                w_f8[name] = w8
            # Wk via casting SWDGE (needed only by the late V projection)
            wk8 = wpool.tile([P, DT, U], f8, tag="w8_k", name="w8_k")
            for t in range(DT):
                nc.gpsimd.dma_start(wk8[:, t, :],
                                    wk_ext[t * P:(t + 1) * P, :])
            w_f8["k"] = wk8

            xb = [xbp.tile([P, D], bf16, tag="xb", name=f"xb{st}")
                  for st in range(ST)]
            xt8 = persist.tile([P, DT, S], f8, tag="xt8", name="xt8")
            qT = [persist.tile([P, S], bf16, tag=f"qT{u}", name=f"qT{u}")
                  for u in range(UT)]
            kT = [persist.tile([P, S], bf16, tag=f"kT{u}", name=f"kT{u}")
                  for u in range(UT)]
            vt = [persist.tile([P, U], bf16, tag=f"v{st}", name=f"v{st}")
                  for st in range(ST)]

            # ---- per chunk: X^T transposes then Q/K projections, so the
            # PE fills DMA-wait gaps with projection work ----
            for c in range(SC):
                for st in range(4 * c, 4 * c + 4):
                    nc.vector.tensor_copy(out=xb[st][:], in_=xf[st][:])
                    for d in range(DT):
                        ps = tps.tile([P, P], bf16, tag="tps")
                        nc.tensor.transpose(
                            ps[:], xb[st][:, d * P:(d + 1) * P], ident[:])
                        nc.vector.tensor_copy(
                            out=xt8[:, d, st * P:(st + 1) * P], in_=ps[:])

                cs = slice(c * 512, (c + 1) * 512)
                for u in range(UT):
                    for dst, wkey in ((qT, "q"), (kT, "v")):
                        ps = mpsum.tile([P, 512], f32, tag="mpsum")
                        for ki in range(0, DT, 2):
                            nc.tensor.matmul(
                                ps[:],
                                w_f8[wkey][:, ki:ki + 2, u * P:(u + 1) * P],
                                xt8[:, ki:ki + 2, cs],
                                start=(ki == 0), stop=(ki == DT - 2),
                                perf_mode=DR)
                        nc.scalar.activation(out=dst[u][:, cs], in_=ps[:],
                                             func=AF.Sigmoid)

            # ---- V projection (waits on the SWDGE Wk load) ----
            for st in range(ST):
                    ps = mpsum.tile([P, U], f32, tag="mpsum")
                    for ki in range(0, DT, 2):
                        nc.tensor.matmul(
                            ps[:],
                            xt8[:, ki:ki + 2, st * P:(st + 1) * P],
                            w_f8["k"][:, ki:ki + 2, :],
                            start=(ki == 0), stop=(ki == DT - 2),
                            perf_mode=DR)
                    nc.scalar.activation(out=vt[st][:], in_=ps[:],
                                         func=AF.Sigmoid)

            # ---- attention row-tiles ----
            for i in range(ST):
                    width = (i + 1) * P  # keys [0, width)
                    nchunk = (width + 511) // 512
                    p_i = pp.tile([P, S], bf16, tag="p")

                    # scores + exp per <=512-wide key chunk; the exp's
                    # accum_out gives the softmax denominator for free
                    partials = []
                    for kc in range(nchunk):
                        w = min(512, width - kc * 512)
                        ps = mpsum.tile([P, 512], f32, tag="mpsum")
                        for u in range(UT):
                            nc.tensor.matmul(
                                ps[:, :w],
                                qT[u][:, i * P:(i + 1) * P],
                                kT[u][:, kc * 512:kc * 512 + w],
                                start=(u == 0), stop=(u == UT - 1))
                        if kc == nchunk - 1:
                            # strict-causal additive mask on the diagonal
                            # block, pre-exp so accum_out sums are exact
                            dlo = i * P - kc * 512
                            nc.vector.tensor_add(
                                out=ps[:, dlo:dlo + P],
                                in0=ps[:, dlo:dlo + P], in1=diag_mask[:])
                        part = smallp.tile([P, 1], f32, tag="part",
                                           name=f"part_{i}_{kc}")
                        nc.scalar.activation(
                            out=p_i[:, kc * 512:kc * 512 + w],
                            in_=ps[:, :w], func=AF.Exp, scale=inv_sqrt_d,
                            accum_out=part[:])
                        partials.append(part)

                    # denominator and its reciprocal
                    denom = smallp.tile([P, 1], f32, tag="denom")
                    # row 0 of tile 0 is fully masked: keep its output at 0
                    nc.vector.tensor_scalar_add(denom[:], partials[0][:],
                                                1e-30)
                    for part in partials[1:]:
                        nc.vector.tensor_add(out=denom[:], in0=denom[:],
                                             in1=part[:])
                    recip = smallp.tile([P, 1], f32, tag="recip")
                    nc.vector.reciprocal(recip[:], denom[:])

                    # P @ V with PE-transposed P blocks (copies on DVE)
                    po = opsum.tile([P, U], f32, tag="opsum")
                    for j in range(i + 1):
                        tp = tps.tile([P, P], bf16, tag="tps")
                        nc.tensor.transpose(tp[:], p_i[:, j * P:(j + 1) * P],
                                            ident[:])
                        pt = ptp.tile([P, P], bf16, tag="pt")
                        nc.vector.tensor_copy(out=pt[:], in_=tp[:])
                        nc.tensor.matmul(po[:], pt[:], vt[j][:],
                                         start=(j == 0), stop=(j == i))

                    # normalize rows on the way out (ACT copy with
                    # per-partition reciprocal scale)
                    ot = outp.tile([P, U], f32, tag="out")
                    nc.scalar.activation(out=ot[:], in_=po[:], func=AF.Copy,
                                         scale=recip[:, 0:1])
                    nc.sync.dma_start(out_ext[i * P:(i + 1) * P, :], ot[:])

    nc.compile()
    return nc


def _get_nc():
    if "nc" not in _cache:
        _cache["nc"] = _build()
    return _cache["nc"]


def kernel(query, Wq, Wv, Wk):
    from concourse.bass_utils import run_bass_kernel_spmd

    nc = _get_nc()
    query = np.ascontiguousarray(query, dtype=np.float32)
    Wq = np.ascontiguousarray(Wq, dtype=np.float32)
    Wv = np.ascontiguousarray(Wv, dtype=np.float32)
    Wk = np.ascontiguousarray(Wk, dtype=np.float32)

    in_maps = [
        {"query": query[b], "Wq": Wq, "Wv": Wv, "Wk": Wk} for b in range(B)
    ]
    res = run_bass_kernel_spmd(nc, in_maps, core_ids=list(range(NCORES)))
    out = np.stack([np.asarray(res.results[b]["out"]) for b in range(B)])
    return out.astype(np.float32)
